# revision 11
# baseline (speedup 1.0000x reference)
"""EPMoE (top-2, 16 experts) forward on 8 Trainium2 NeuronCores.

Strategy (expert parallel):
  - Host: router softmax/top-2/renorm, token->expert dispatch (stable order,
    matching the reference), weight re-layout into slab-contiguous form,
    final weighted combine.
  - Device (per core, 2 expert slots): grouped GEMM1 [W,H]x[H,2I] ->
    silu*up -> grouped GEMM2 [W,I]x[I,H], all matmuls bf16 at full PE
    rate, weights streamed from HBM as large fully-contiguous slabs
    (memory-bound roofline).

Load balancing: experts are sorted by dispatched-token count and paired
(hottest with coldest) onto cores; the two slots get compile-time column
widths W0/W1 = the slot-wise maxima (padded to 8).  This keeps the SPMD
program uniform across cores while cutting PE streaming work vs a single
global capacity, so every phase stays DMA-bound and the weight stream
never stalls behind the PE.

The reference's simulated fp8 quantization (amax scaling + clip, no
rounding) cancels exactly: (x/sa) @ (w/sw)^T * sa*sw == x @ w^T, and the
+-448 clip never binds for amax-scaled values.  So the kernel computes
the plain MoE forward.
"""

import ml_dtypes
import numpy as np

import concourse.bass as bass
import concourse.bacc as bacc
import concourse.mybir as mybir
import concourse.tile as tile
from concourse.bass_utils import run_bass_kernel_spmd

dt = mybir.dt

# Problem shape (hardcoded per spec)
T, H, I, E, TOP_K = 1024, 2048, 1408, 16, 2
TWO_I = 2 * I
N_CORES = 8
EPC = E // N_CORES  # expert slots per core
CAP = 512           # reference capacity: in-expert position >= CAP dropped

KT1 = H // 128      # 16 contraction tiles for GEMM1
FT = I // 128       # 11 feature tiles per gate/up half
KT2 = I // 128      # 11 contraction tiles for GEMM2
MT_GRP = 2          # number of GEMM2 m-groups
MT_G = H // 128 // MT_GRP  # 8 output tiles per m-group
MW = MT_G * 128     # 1024, m-group output width
KB1 = 4             # k-tiles per GEMM1 weight slab (one DMA)

# bf16 halves the HBM weight traffic (the memory-bound term) vs f32r and
# keeps the output error ~5e-3, far under the 2e-2 gate.
USE_BF16 = True

_PROGRAMS = {}  # (W0, W1) -> compiled Bacc program


def _build_program(W0, W1):
    """One SPMD program: per core, 2 expert slots' MoE FFN over W0/W1
    padded token columns.

    DRAM layouts are slab-contiguous (host pre-arranged):
      w13t[s, fh, kh, p, kk, c] = w13[g, fh*I + c, 128*(4kh+kk) + p]
      w2t [s, mg, p, k2, c]    = w2[g, mg*MW + c, 128*k2 + p]
      xt{s}[p, k, c]           = x[token c of slot s, 128k + p]
      yt{s}[mg, p, m, c]       = y^T[h = mg*MW + 128m + p, token c]
    """
    nc = bacc.Bacc("TRN2", target_bir_lowering=False, debug=False,
                   num_devices=N_CORES)

    wdt = dt.bfloat16 if USE_BF16 else dt.float32r
    ydt = dt.bfloat16 if USE_BF16 else dt.float32
    w13t = nc.declare_dram_parameter("w13t", [EPC, 2, KT1 // KB1, 128, KB1, I],
                                     wdt, isOutput=False)
    w2t = nc.declare_dram_parameter("w2t", [EPC, MT_GRP, 128, KT2, MW],
                                    wdt, isOutput=False)
    xts = [nc.declare_dram_parameter(f"xt{s}", [128, KT1, W], wdt,
                                     isOutput=False)
           for s, W in enumerate((W0, W1))]
    yts = [nc.declare_dram_parameter(f"yt{s}", [MT_GRP, 128, MT_G, W], ydt,
                                     isOutput=True)
           for s, W in enumerate((W0, W1))]

    silu_fn = mybir.ActivationFunctionType.Silu
    copy_fn = mybir.ActivationFunctionType.Copy

    with tile.TileContext(nc) as tc:
        with (
            tc.tile_pool(name="xpool", bufs=2) as xpool,
            tc.tile_pool(name="w1pool", bufs=6) as w1pool,
            tc.tile_pool(name="w2pool", bufs=4) as w2pool,
            tc.tile_pool(name="spool", bufs=FT + 1) as spool,
            tc.tile_pool(name="apool", bufs=2 * FT) as apool,
            tc.tile_pool(name="ypool", bufs=4) as ypool,
            tc.tile_pool(name="psum", bufs=8, space="PSUM") as pspool,
        ):
            # Issue BOTH slots' x loads up front: the gpsimd queue is
            # in-order, and slot0's y-write pushes would otherwise delay
            # slot1's x until slot0 fully drains.  First-kh piece first so
            # the first matmuls can start before the whole tile lands.
            xtes = []
            for s, W in enumerate((W0, W1)):
                xte = xpool.tile([128, KT1, W], wdt, tag=f"xte{s}")
                nc.gpsimd.dma_start(xte[:, :KB1, :], xts[s][:, :KB1, :])
                nc.gpsimd.dma_start(xte[:, KB1:, :], xts[s][:, KB1:, :])
                xtes.append(xte)

            for s, W in enumerate((W0, W1)):
                xte = xtes[s]

                # ---- GEMM1 + silu_and_mul ----
                silu_tiles = []
                act_tiles = []
                for fh in range(2):  # 0 = gate half, 1 = up half
                    # Pack THREE column groups per 2KB PSUM bank
                    # (3*W*4B <= 2048 for W <= 170): each GEMM1 half uses
                    # 4 banks, so phases alternate between two clean
                    # 4-bank sets and never block on a previous phase's
                    # drain.
                    pst = [pspool.tile([128, 3 * W], dt.float32, tag="ps",
                                       name=f"ps1_{s}_{fh}_{i}")
                           for i in range((FT + 2) // 3)]
                    for kh in range(KT1 // KB1):
                        slab = w1pool.tile([128, KB1, I], wdt, tag="w13")
                        # Sub-slab DMA pieces: matmuls start on partial
                        # slabs (subtile deps), smoothing slab-edge stalls;
                        # finest pieces on the very first phase so the PE
                        # pipeline fills as early as possible.
                        np_pieces = KB1 if (s == 0 and fh == 0) else 2
                        step = KB1 // np_pieces
                        for pi in range(np_pieces):
                            lo = pi * step
                            if s == 0 and fh == 0 and kh == 0 and pi == 0:
                                # halve the very first piece along the
                                # feature dim: the first matmuls need only
                                # the first few 128-wide weight tiles.
                                nc.sync.dma_start(
                                    slab[:, :1, :I // 2],
                                    w13t[s, fh, kh, :, :1, :I // 2])
                                nc.sync.dma_start(
                                    slab[:, :1, I // 2:],
                                    w13t[s, fh, kh, :, :1, I // 2:])
                                continue
                            nc.sync.dma_start(slab[:, lo:lo + step, :],
                                              w13t[s, fh, kh, :, lo:lo + step, :])
                        for kk in range(KB1):
                            k = kh * KB1 + kk
                            for j in range(FT):
                                dst = pst[j // 3][:,
                                                  (j % 3) * W:(j % 3 + 1) * W]
                                # start=True clears has_written for the WHOLE
                                # bank: only the first group packed into each
                                # bank may set it.  Later groups' k==0
                                # matmuls overwrite (their bits are clear).
                                nc.tensor.matmul(
                                    dst,
                                    slab[:, kk, j * 128:(j + 1) * 128],
                                    xte[:, k, :],
                                    start=(k == 0 and j % 3 == 0),
                                    stop=(k == KT1 - 1),
                                    skip_group_check=(j % 3 != 0),
                                )
                    for j in range(FT):
                        src = pst[j // 3][:, (j % 3) * W:(j % 3 + 1) * W]
                        if fh == 0:
                            st = spool.tile([128, W], wdt, tag="silu",
                                            name=f"silu_{s}_{j}")
                            nc.scalar.activation(st[:], src, silu_fn)
                            silu_tiles.append(st)
                        else:
                            at = apool.tile([128, W], wdt, tag="act",
                                            name=f"act_{s}_{j}")
                            nc.vector.tensor_mul(at[:], silu_tiles[j][:], src)
                            act_tiles.append(at)

                # ---- GEMM2 ----
                for mg in range(MT_GRP):
                    pst2 = [pspool.tile([128, 2 * W], dt.float32, tag="ps",
                                        name=f"ps2_{s}_{mg}_{i}")
                            for i in range(MT_G // 2)]
                    slab2 = w2pool.tile([128, KT2, MW], wdt, tag="w2")
                    for lo, hi in ((0, 4), (4, 8), (8, 10), (10, KT2)):
                        nc.sync.dma_start(slab2[:, lo:hi, :],
                                          w2t[s, mg, :, lo:hi, :])
                    for k2 in range(KT2):
                        for m in range(MT_G):
                            dst = pst2[m // 2][:, (m % 2) * W:(m % 2 + 1) * W]
                            nc.tensor.matmul(
                                dst,
                                slab2[:, k2, m * 128:(m + 1) * 128],
                                act_tiles[k2][:],
                                start=(k2 == 0 and m % 2 == 0),
                                stop=(k2 == KT2 - 1),
                                skip_group_check=(m % 2 == 1),
                            )
                    # Drain PSUM with vector AND scalar engines in
                    # parallel; write y out with few, large-run DMA
                    # descriptors (small packets waste shared DMA-engine
                    # time and starve the weight stream).
                    ybig = ypool.tile([128, MT_G, W], ydt, tag="y",
                                      name=f"y_{s}_{mg}")
                    for m in range(MT_G):
                        src = pst2[m // 2][:, (m % 2) * W:(m % 2 + 1) * W]
                        if m % 2 == 0:
                            nc.vector.tensor_copy(ybig[:, m, :], src)
                        else:
                            nc.scalar.activation(ybig[:, m, :], src, copy_fn)
                    half = MT_G // 2
                    if s == 1 and mg == MT_GRP - 1:
                        # drain queues are empty at the end: use the fast
                        # HW queues, split for latency.
                        nc.sync.dma_start(yts[s][mg, :, :half, :],
                                          ybig[:, :half, :])
                        nc.scalar.dma_start(yts[s][mg, :, half:, :],
                                            ybig[:, half:, :])
                    else:
                        nc.gpsimd.dma_start(yts[s][mg], ybig[:])

    nc.compile()
    return nc


def _get_program(W0, W1):
    key = (W0, W1)
    if key not in _PROGRAMS:
        _PROGRAMS[key] = _build_program(W0, W1)
    return _PROGRAMS[key]


def _route(router_logits):
    """Replicate the reference routing in numpy (fp32)."""
    lm = router_logits - router_logits.max(axis=-1, keepdims=True)
    p = np.exp(lm)
    probs = p / p.sum(axis=-1, keepdims=True)
    topi = np.argsort(-probs, axis=-1, kind="stable")[:, :TOP_K]
    topw = np.take_along_axis(probs, topi, axis=-1)
    topw = topw / topw.sum(axis=-1, keepdims=True)

    rid = topi.reshape(-1)
    rtok = np.arange(T * TOP_K) // TOP_K
    order = np.argsort(rid, kind="stable")
    counts = np.bincount(rid, minlength=E)
    offsets = np.concatenate([[0], np.cumsum(counts)[:-1]])
    return topw, rid, rtok, order, counts, offsets


def _plan(counts):
    """Pair hot and cold experts onto cores; slot widths = slot maxima.

    Returns (perm, W0, W1, n_chunks): perm[2c+s] is the expert id served
    by core c slot s; slot s processes chunks of W_s token columns.
    """
    capped = np.minimum(counts, CAP).astype(np.int64)
    ranks = np.argsort(-capped, kind="stable")
    perm = np.empty(E, np.int64)
    for c in range(N_CORES):
        perm[2 * c] = ranks[c]
        perm[2 * c + 1] = ranks[E - 1 - c]

    def pad8(n):
        return max(8, -(-int(n) // 8) * 8)

    # PSUM packs three column groups per 2KB bank: need 3*W <= 512 fp32.
    n_chunks = max(1, -(-int(capped.max()) // 168))
    W0 = pad8(-(-int(capped[ranks[0]]) // n_chunks))
    W1 = pad8(-(-int(capped[ranks[N_CORES]]) // n_chunks))
    return perm, W0, W1, n_chunks


def _np_wdt():
    return ml_dtypes.bfloat16 if USE_BF16 else np.float32


def _prep_weights(w13_weight, w2_weight, perm):
    """Slab-contiguous per-core weight shards (see _build_program)."""
    w13t_cores, w2t_cores = [], []
    for c in range(N_CORES):
        a = np.empty((EPC, 2, KT1 // KB1, 128, KB1, I), _np_wdt())
        b = np.empty((EPC, MT_GRP, 128, KT2, MW), _np_wdt())
        for el in range(EPC):
            g = int(perm[c * EPC + el])
            # [H, 2I] -> (kh, kk, p, fh, c) -> (fh, kh, p, kk, c)
            a[el] = (
                w13_weight[g].T
                .reshape(KT1 // KB1, KB1, 128, 2, I)
                .transpose(3, 0, 2, 1, 4)
            )
            # [I, H] -> (k2, p, mg, c) -> (mg, p, k2, c)
            b[el] = (
                w2_weight[g].T.reshape(KT2, 128, MT_GRP, MW)
                .transpose(2, 1, 0, 3)
            )
        w13t_cores.append(a)
        w2t_cores.append(b)
    return w13t_cores, w2t_cores


def _make_in_maps(x, w13t_cores, w2t_cores, expert_rows, rtok, perm,
                  W0, W1, chunk):
    """Per-core input maps for one capacity chunk."""
    in_maps = []
    for c in range(N_CORES):
        m = {"w13t": w13t_cores[c], "w2t": w2t_cores[c]}
        for s, W in enumerate((W0, W1)):
            g = int(perm[c * EPC + s])
            xt_s = np.zeros((128, KT1, W), _np_wdt())
            rows = expert_rows[g][chunk * W:(chunk + 1) * W]
            if len(rows):
                # [n, H] -> [H, n] -> [KT1, 128, n] -> [128, KT1, n]
                xt_s[:, :, :len(rows)] = (
                    x[rtok[rows]].T.reshape(KT1, 128, -1).transpose(1, 0, 2)
                )
            m[f"xt{s}"] = xt_s
        in_maps.append(m)
    return in_maps


def kernel(x, router_logits, w13_weight, w2_weight):
    x = np.asarray(x, dtype=np.float32)
    router_logits = np.asarray(router_logits, dtype=np.float32)
    w13_weight = np.asarray(w13_weight, dtype=np.float32)
    w2_weight = np.asarray(w2_weight, dtype=np.float32)
    assert x.shape == (T, H) and router_logits.shape == (T, E)
    assert w13_weight.shape == (E, TWO_I, H) and w2_weight.shape == (E, H, I)

    topw, rid, rtok, order, counts, offsets = _route(router_logits)
    perm, W0, W1, n_chunks = _plan(counts)
    eff = int(min(counts.max(), CAP))

    nc = _get_program(W0, W1)
    w13t_cores, w2t_cores = _prep_weights(w13_weight, w2_weight, perm)

    # token rows per expert, in reference (stable) dispatch order,
    # truncated to capacity
    expert_rows = [
        order[offsets[g]:offsets[g] + min(int(counts[g]), CAP)]
        for g in range(E)
    ]

    ybuf = np.zeros((E, eff, H), np.float32)

    def _run_chunk(chunk):
        in_maps = _make_in_maps(x, w13t_cores, w2t_cores, expert_rows,
                                rtok, perm, W0, W1, chunk)
        res = run_bass_kernel_spmd(nc, in_maps, list(range(N_CORES)))
        for c in range(N_CORES):
            for s, W in enumerate((W0, W1)):
                g = int(perm[c * EPC + s])
                n = len(expert_rows[g][chunk * W:(chunk + 1) * W])
                if n:
                    lo = chunk * W
                    yt_s = res.results[c][f"yt{s}"]  # [MT_GRP,128,MT_G,W]
                    # -> y^T [H, W] -> [n, H]
                    ytr = (
                        yt_s.transpose(0, 2, 1, 3).reshape(H, W)
                    ).astype(np.float32)
                    ybuf[g, lo:lo + n] = ytr[:, :n].T

    def _spot_ok(chunk):
        # one token per expert vs numpy fp32: catches rare flaky-device
        # corruption (bf16 path error is ~5e-3, far under the gate)
        for c in range(N_CORES):
            for s, W in enumerate((W0, W1)):
                g = int(perm[c * EPC + s])
                rows = expert_rows[g][chunk * W:(chunk + 1) * W]
                if not len(rows):
                    continue
                tok = rtok[rows[0]]
                h = x[tok] @ w13_weight[g].T
                act = h[:I] / (1.0 + np.exp(-h[:I])) * h[I:]
                yref = act @ w2_weight[g].T
                got = ybuf[g, chunk * W]
                if np.linalg.norm(got - yref) > 0.05 * np.linalg.norm(yref):
                    return False
        return True

    for chunk in range(n_chunks):
        _run_chunk(chunk)
        if not _spot_ok(chunk):
            _run_chunk(chunk)  # one retry on a flaky device result

    # ---- combine: gather rows back, weight by router probs ----
    pos = np.empty(T * TOP_K, np.int64)
    for g in range(E):
        pos[order[offsets[g]:offsets[g] + counts[g]]] = np.arange(counts[g])
    valid = (pos < CAP).astype(np.float32)
    posc = np.minimum(pos, eff - 1)
    yrows = ybuf[rid, posc] * valid[:, None]  # [T*K, H]
    out = np.einsum(
        "tkh,tk->th", yrows.reshape(T, TOP_K, H), topw.astype(np.float32)
    )
    return out.astype(np.float32)
